# revision 23
# baseline (speedup 1.0000x reference)
"""Chamfer loss kernel for Trainium2 (8 NeuronCores, Bass/Tile).

Problem: x (4, 8192, 3), y (4, 8192, 3) fp32.
  dist[b,i,j] = ||x_bi||^2 + ||y_bj||^2 - 2 x_bi . y_bj
  out = mean_b( mean_i min_j dist + mean_j min_i dist )

Sharding: 8 cores = 4 batches x 2 halves. Core (b, h) computes
  - x->y mins for x rows [h*4096, (h+1)*4096) of batch b vs ALL y[b]
  - y->x mins for y rows [h*4096, (h+1)*4096) of batch b vs ALL x[b]
so no cross-core reduction is needed (each core owns full rows of output).

On-chip compute: G[i,j] = -2 q_i . d_j + ||d_j||^2 via a single K=21 bf16
matmul using 3-term hi/mid/lo splitting (beyond-fp32 accuracy at bf16 matmul
speed; matmul cost depends only on the free dim, not K):
  q = A + AL + AL2 (+ 2^-27),  -2d = C + E + E2,  ||d||^2 = d2h + d2l + d2l2
  G = A.(C+E+E2) + AL.(C+E) + AL2.C + d2h + d2l + d2l2
Then min_j dist = ||q_i||^2 + min_j G[i,j]; the min runs on VectorE from PSUM.
The ||q_i||^2 add + means happen on the host in float64 (cheap: O(N)).

Drain pipeline (the bottleneck: every PSUM element crosses DVE@0.96GHz or
ACT@1.2GHz once): PSUM groups of 1024 fp32 (2 banks) are processed in pairs:
ScalarE copies group 2k to SBUF, VectorE runs tensor_tensor_scan
(state = min(min(psum_grp_{2k+1}[t], state), sbuf_copy[t])) whose final
column is the running min of BOTH groups; the scan state chains across pairs
via `initial`, so one block (8192 db points) ends as a single [128,1] column.
This halves DVE element traffic vs per-group tensor_reduce.
(tensor_tensor_reduce would fuse the same thing but is broken on HW here.)
"""

import numpy as np
import ml_dtypes

B = 4
N = 8192  # x points per batch
M = 8192  # y points per batch
D = 3
NCORES = 8

QROWS = 4096  # query rows per core (half of a batch's points)
DBN = 8192  # database points scanned per query
KDIM = 21  # augmented contraction dim
BLKP = 128  # query rows per matmul block (PSUM partitions)
FREE = 512  # matmul free size (one PSUM fp32 bank)
GROUP = 1024  # PSUM group per drain op (2 banks)
NPAIR = DBN // (2 * GROUP)  # group-pairs per block -> output cols per block

_NC_CACHE = {}


def _build_nc(qrows=QROWS, dbn=DBN):
    """Build + compile the (SPMD, identical on all cores) Bass program."""
    from contextlib import ExitStack

    import concourse.tile as tile
    from concourse import bacc, mybir

    bf16 = mybir.dt.bfloat16
    f32 = mybir.dt.float32

    nblk = qrows // BLKP
    npair = dbn // (2 * GROUP)
    outc = nblk

    nc = bacc.Bacc(
        "TRN2", target_bir_lowering=False, debug=False, num_devices=NCORES
    )
    lx = nc.dram_tensor("lx", [KDIM, qrows], bf16, kind="ExternalInput")
    ry = nc.dram_tensor("ry", [KDIM, dbn], bf16, kind="ExternalInput")
    ly = nc.dram_tensor("ly", [KDIM, qrows], bf16, kind="ExternalInput")
    rx = nc.dram_tensor("rx", [KDIM, dbn], bf16, kind="ExternalInput")
    ox = nc.dram_tensor("ox", [BLKP, outc], f32, kind="ExternalOutput")
    oy = nc.dram_tensor("oy", [BLKP, outc], f32, kind="ExternalOutput")

    with tile.TileContext(nc) as tc, ExitStack() as ctx:
        cpool = ctx.enter_context(tc.tile_pool(name="consts", bufs=1))
        psum_bufs = 16384 // (GROUP * 4)  # fill all 8 PSUM banks
        ppool = ctx.enter_context(
            tc.tile_pool(name="psum", bufs=psum_bufs, space="PSUM")
        )
        spool = ctx.enter_context(tc.tile_pool(name="scratch", bufs=3))
        wpool = ctx.enter_context(tc.tile_pool(name="waste", bufs=2))
        opool = ctx.enter_context(tc.tile_pool(name="outs", bufs=1))

        s_lx = cpool.tile([KDIM, qrows], bf16, tag="lx")
        s_ry = cpool.tile([KDIM, dbn], bf16, tag="ry")
        s_ly = cpool.tile([KDIM, qrows], bf16, tag="ly")
        s_rx = cpool.tile([KDIM, dbn], bf16, tag="rx")
        nc.sync.dma_start(s_lx[:], lx[:])
        nc.sync.dma_start(s_ry[:], ry[:])
        nc.sync.dma_start(s_ly[:], ly[:])
        nc.sync.dma_start(s_rx[:], rx[:])

        s_ox = opool.tile([BLKP, outc], f32, tag="ox")
        s_oy = opool.tile([BLKP, outc], f32, tag="oy")

        def fill_group(lhs_blk, s_r, g):
            """Emit matmuls computing G for db cols [g*GROUP, (g+1)*GROUP)."""
            ps = ppool.tile([BLKP, GROUP], f32, tag="ps")
            for t in range(GROUP // FREE):
                col0 = g * GROUP + t * FREE
                nc.tensor.matmul(
                    ps[:, t * FREE : (t + 1) * FREE],
                    lhs_blk,
                    s_r[:, col0 : col0 + FREE],
                    start=True,
                    stop=True,
                )
            return ps

        for s_l, s_r, s_o, o_dram in (
            (s_lx, s_ry, s_ox, ox),
            (s_ly, s_rx, s_oy, oy),
        ):
            for blk in range(nblk):
                lhs_blk = s_l[:, blk * BLKP : (blk + 1) * BLKP]
                prev = None
                for p in range(npair):
                    ps_a = fill_group(lhs_blk, s_r, 2 * p)
                    sb_a = spool.tile([BLKP, GROUP], f32, tag="sb")
                    nc.scalar.copy(sb_a[:], ps_a[:])
                    ps_b = fill_group(lhs_blk, s_r, 2 * p + 1)
                    waste = wpool.tile([BLKP, GROUP], f32, tag="w")
                    init = (
                        float(np.finfo(np.float32).max)
                        if prev is None
                        else prev[:, GROUP - 1 : GROUP]
                    )
                    nc.vector.tensor_tensor_scan(
                        waste[:],
                        ps_b[:],
                        sb_a[:],
                        initial=init,
                        op0=mybir.AluOpType.min,
                        op1=mybir.AluOpType.min,
                    )
                    prev = waste
                nc.vector.tensor_copy(
                    s_o[:, blk : blk + 1], prev[:, GROUP - 1 : GROUP]
                )
            nc.sync.dma_start(o_dram[:], s_o[:])

    nc.compile()
    return nc


def _get_nc(qrows=QROWS, dbn=DBN):
    key = (qrows, dbn)
    if key not in _NC_CACHE:
        _NC_CACHE[key] = _build_nc(qrows, dbn)
    return _NC_CACHE[key]


def _split3(a):
    """fp32 array -> (hi, mid, lo) bf16 triple, hi+mid+lo ~ a to ~2^-27 |a|."""
    hi = a.astype(ml_dtypes.bfloat16)
    r = a - hi.astype(np.float32)
    mid = r.astype(ml_dtypes.bfloat16)
    lo = (r - mid.astype(np.float32)).astype(ml_dtypes.bfloat16)
    return hi, mid, lo


def _build_lhs(q):
    """q [Q, 3] fp32 -> stationary operand [21, Q] bf16."""
    qq = np.ascontiguousarray(q.T)  # [3, Q]
    A, AL, AL2 = _split3(qq)
    ones = np.ones((3, q.shape[0]), dtype=ml_dtypes.bfloat16)
    return np.concatenate([A, A, A, AL, AL, AL2, ones], axis=0)


def _build_rhs(d):
    """d [Dn, 3] fp32 -> moving operand [21, Dn] bf16."""
    t = np.ascontiguousarray(d.T) * np.float32(-2.0)  # [3, Dn]
    C, E, E2 = _split3(t)
    d2 = (d.astype(np.float64) ** 2).sum(axis=1).astype(np.float32)[None, :]
    d2h, d2l, d2l2 = _split3(d2)
    return np.concatenate([C, E, E2, C, E, C, d2h, d2l, d2l2], axis=0)


def _unpack_mins(o):
    """o [128, nblk] fp32 per-block G-mins -> [nblk*128] row G-mins."""
    return np.asarray(o).T.reshape(-1)  # row = blk*128 + p


def kernel(x, y):
    from concourse.bass_utils import run_bass_kernel_spmd

    x = np.asarray(x, dtype=np.float32)
    y = np.asarray(y, dtype=np.float32)
    assert x.shape == (B, N, D) and y.shape == (B, M, D)

    in_maps = []
    rhs_y = [_build_rhs(y[b]) for b in range(B)]
    rhs_x = [_build_rhs(x[b]) for b in range(B)]
    for c in range(NCORES):
        b, h = divmod(c, 2)
        sl = slice(h * QROWS, (h + 1) * QROWS)
        in_maps.append(
            {
                "lx": _build_lhs(x[b, sl]),
                "ry": rhs_y[b],
                "ly": _build_lhs(y[b, sl]),
                "rx": rhs_x[b],
            }
        )

    nc = _get_nc()
    res = run_bass_kernel_spmd(nc, in_maps, core_ids=list(range(NCORES)))

    total = 0.0
    for b in range(B):
        x2 = (x[b].astype(np.float64) ** 2).sum(axis=1)  # [N]
        y2 = (y[b].astype(np.float64) ** 2).sum(axis=1)  # [M]
        minx = np.empty(N, dtype=np.float64)
        miny = np.empty(M, dtype=np.float64)
        for h in range(2):
            r = res.results[2 * b + h]
            sl = slice(h * QROWS, (h + 1) * QROWS)
            minx[sl] = _unpack_mins(r["ox"])
            miny[sl] = _unpack_mins(r["oy"])
        minx += x2
        miny += y2
        total += minx.mean() + miny.mean()

    return np.float32(total / B)
